# revision 20
# baseline (speedup 1.0000x reference)
"""Trainium2 Bass kernel for nn_ColourHistogram.

Computes, per image, hist = ka^T @ kb where ka/kb are Gaussian
soft-assignment matrices [HW, 32] built from the two channels:
    ka[p, a] = exp(-0.5*((x_p - a/31)/sigma)^2) = exp(-C*(31*x_p - a)^2)
with C = 0.5/(31*sigma)^2.

Strategy (pure data parallel, 2 images per core on 8 cores):
  - pixels live on the 128 SBUF partitions, pixel-columns on the free dim
  - squared-distance planes u_a = (31x - a)^2 are produced on the DVE with
    a per-bin recurrence  u_{a+1} = (u_a + (2a+1)) + (-62x)
  - one ACT pass per bin-group applies exp(-C*u) -> fp16 weights
  - the PE contracts ka^T kb over pixels, accumulating in PSUM
"""

import numpy as np
from contextlib import ExitStack

import jax

try:
    jax.config.update("jax_compilation_cache_dir", "/tmp/jaxcache")
    jax.config.update("jax_persistent_cache_min_compile_time_secs", 1.0)
except Exception:
    pass

import concourse.bass as bass
import concourse.bacc as bacc
import concourse.tile as tile
from concourse import mybir
from concourse.bass_utils import run_bass_kernel_spmd

BINS = 32
SIGMA = 0.05
N_CORES = 8
IMGS = 2          # images per core
H = W = 512
P = 128
COLS = (H * W) // P  # 2048 pixel-columns per channel image
C_EXP = 0.5 / (31.0 * SIGMA) ** 2  # 0.20811654526...

F32 = mybir.dt.float32
F16 = mybir.dt.float16

AF = mybir.ActivationFunctionType
OP = mybir.AluOpType


def build_bass(cols=COLS, slab=512, grp=4, imgs=IMGS, reps=1,
               gen="stt", pack=True, stream_x=False, ubufs=2, kbufs=2):
    """Build the per-core Bass program. Shapes are per-core.

    reps > 1 wraps the computation in a hardware loop so device time can
    be measured as (wall(R) - wall(1)) / (R - 1).
    gen: "stt" uses the fused scalar_tensor_tensor recurrence;
         "native" uses tensor_scalar + tensor_tensor pairs.
    pack: True packs 4 pixel-chunks into [128,128] matmuls (kt layout
          [P, slab, BINS], strided exp writes); False uses one chunk per
          matmul (kt layout [P, BINS, slab], dense exp writes).
    """
    n_slabs = cols // slab
    assert cols % slab == 0 and BINS % grp == 0 and slab % 4 == 0

    nc = bacc.Bacc("TRN2", debug=False)
    image = nc.dram_tensor(
        "image", [imgs, 2, P * cols], F32, kind="ExternalInput"
    )
    hist = nc.dram_tensor(
        "hist", [imgs, 1, BINS, BINS], F32, kind="ExternalOutput"
    )

    with ExitStack() as ctx:
        tc = ctx.enter_context(tile.TileContext(nc))

        xpool = ctx.enter_context(tc.tile_pool(name="x", bufs=1 if not stream_x else 3))
        n2pool = ctx.enter_context(tc.tile_pool(name="n2", bufs=2))
        upool = ctx.enter_context(tc.tile_pool(name="u", bufs=ubufs))
        kpool = ctx.enter_context(tc.tile_pool(name="k", bufs=kbufs))
        pspool = ctx.enter_context(tc.tile_pool(name="ps", bufs=1, space="PSUM"))
        fpool = ctx.enter_context(tc.tile_pool(name="fin", bufs=1))

        def gen_channel(xs, kt):
            """Fill kt with exp(-C*(31*xs - a)^2) for a = 0..31."""
            if gen == "stt":
                n2 = n2pool.tile([P, slab], F32, tag="n2")
                nc.scalar.mul(n2, xs, -62.0)
            uprev = None
            for g in range(BINS // grp):
                ug = upool.tile([P, grp, slab], F32, tag="u")
                for j in range(grp):
                    a = g * grp + j
                    if a == 0:
                        nc.scalar.activation(
                            out=ug[:, 0, :], in_=xs,
                            func=AF.Square, scale=31.0,
                        )
                        continue
                    usrc = uprev[:, grp - 1, :] if j == 0 else ug[:, j - 1, :]
                    if gen == "stt":
                        # u_a = (u_{a-1} + (2a-1)) + (-62 x)
                        nc.vector.scalar_tensor_tensor(
                            out=ug[:, j, :], in0=usrc,
                            scalar=float(2 * a - 1), in1=n2,
                            op0=OP.add, op1=OP.add,
                        )
                    else:
                        d = n2pool.tile([P, slab], F32, tag="d")
                        nc.vector.tensor_scalar(
                            out=d, in0=xs, scalar1=-62.0,
                            scalar2=float(2 * a - 1),
                            op0=OP.mult, op1=OP.add,
                        )
                        nc.vector.tensor_add(ug[:, j, :], usrc, d)
                if pack is True:
                    out_v = (kt[:, :, g * grp:(g + 1) * grp]
                             .rearrange("p w a -> p a w"))
                else:
                    out_v = kt[:, g * grp:(g + 1) * grp, :]
                nc.scalar.activation(
                    out=out_v, in_=ug, func=AF.Exp, scale=-C_EXP,
                )
                uprev = ug

        def body():
            # ---- load channel images: x_sb[:, ci, :], ci = img*2 + ch ----
            if not stream_x:
                x_sb = xpool.tile([P, 2 * imgs, cols], F32)
                for img in range(imgs):
                    for ch in range(2):
                        ci = img * 2 + ch
                        src = image[img, ch].rearrange("(p c) -> p c", p=P)
                        nc.sync.dma_start(out=x_sb[:, ci, :], in_=src)

            if pack is True:
                ph = [pspool.tile([P, 4, BINS], F32, name=f"ph{i}")
                      for i in range(imgs)]
            elif pack == "ct":
                ph = [pspool.tile([96, BINS], F32, name=f"ph{i}")
                      for i in range(imgs)]
            else:
                ph = [pspool.tile([BINS, BINS], F32, name=f"ph{i}")
                      for i in range(imgs)]

            kshape = [P, slab, BINS] if pack is True else [P, BINS, slab]
            for img in range(imgs):
                first_mm = True
                for s in range(n_slabs):
                    c0 = s * slab
                    if stream_x:
                        # per-slab channel-pair load (keeps SBUF small)
                        x_sl = xpool.tile([P, 2, slab], F32, tag="xs")
                        src = image[img].rearrange(
                            "c (p w) -> p c w", p=P)[:, :, c0:c0 + slab]
                        nc.sync.dma_start(out=x_sl, in_=src)
                    ksl = []
                    for ch in range(2):
                        ci = img * 2 + ch
                        if stream_x:
                            xs = x_sl[:, ch, :]
                        else:
                            xs = x_sb[:, ci, c0:c0 + slab]  # [128, slab]
                        kt = kpool.tile(kshape, F16, tag=f"k{ch}")
                        ksl.append(kt)
                        gen_channel(xs, kt)

                    if pack is True:
                        # 4 pixel-chunks per LDW+MM pair: [128,128] operands
                        for w in range(0, slab, 4):
                            last = (s == n_slabs - 1) and (w == slab - 4)
                            nc.tensor.matmul(
                                ph[img][:],
                                ksl[0][:, w:w + 4, :],
                                ksl[1][:, w:w + 4, :],
                                start=first_mm, stop=last,
                            )
                            first_mm = False
                    elif pack == "ct":
                        # 4-way column tiling: chunk w runs in col-group
                        # w%4 of the PE array (tile_position auto-derived
                        # from the PSUM slice base partition); the four
                        # 32x32 matmuls execute concurrently
                        for w in range(slab):
                            j = w % 3
                            first = (s == 0) and (w < 3)
                            last = (s == n_slabs - 1) and (w >= slab - 3)
                            nc.tensor.matmul(
                                ph[img][32 * j:32 * (j + 1), :],
                                ksl[0][:, :, w],
                                ksl[1][:, :, w],
                                start=first, stop=last,
                            )
                    else:
                        for w in range(slab):
                            last = (s == n_slabs - 1) and (w == slab - 1)
                            nc.tensor.matmul(
                                ph[img][:],
                                ksl[0][:, :, w],
                                ksl[1][:, :, w],
                                start=first_mm, stop=last,
                            )
                            first_mm = False

            # ---- finalize ----
            for img in range(imgs):
                if pack == "ct":
                    # sum the 3 col-group partials stacked on partitions
                    hsb = fpool.tile([96, BINS], F32, name=f"hsb{img}")
                    nc.vector.tensor_copy(hsb, ph[img][:])
                    dg = fpool.tile([BINS, 3, BINS], F32, name=f"dg{img}")
                    for g in range(3):
                        nc.sync.dma_start(
                            out=dg[:, g, :],
                            in_=hsb[32 * g:32 * (g + 1), :],
                        )
                    t0 = fpool.tile([BINS, BINS], F32, name=f"t0_{img}")
                    t2 = fpool.tile([BINS, BINS], F32, name=f"t2_{img}")
                    nc.vector.tensor_add(t0, dg[:, 0, :], dg[:, 1, :])
                    nc.vector.tensor_add(t2, t0, dg[:, 2, :])
                    nc.sync.dma_start(out=hist[img, 0], in_=t2)
                elif pack is True:
                    # sum the 4 diagonal 32x32 blocks
                    hsb = fpool.tile([P, 4, BINS], F32, name=f"hsb{img}")
                    nc.vector.tensor_copy(hsb, ph[img][:])
                    dg = fpool.tile([BINS, 4, BINS], F32, name=f"dg{img}")
                    for g in range(4):
                        nc.sync.dma_start(
                            out=dg[:, g, :],
                            in_=hsb[32 * g:32 * (g + 1), g, :],
                        )
                    t0 = fpool.tile([BINS, BINS], F32, name=f"t0_{img}")
                    t1 = fpool.tile([BINS, BINS], F32, name=f"t1_{img}")
                    t2 = fpool.tile([BINS, BINS], F32, name=f"t2_{img}")
                    nc.vector.tensor_add(t0, dg[:, 0, :], dg[:, 1, :])
                    nc.vector.tensor_add(t1, dg[:, 2, :], dg[:, 3, :])
                    nc.vector.tensor_add(t2, t0, t1)
                    nc.sync.dma_start(out=hist[img, 0], in_=t2)
                else:
                    t2 = fpool.tile([BINS, BINS], F32, name=f"t2_{img}")
                    nc.vector.tensor_copy(t2, ph[img][:])
                    nc.sync.dma_start(out=hist[img, 0], in_=t2)

        if reps > 1:
            with tc.For_i(0, reps, 1):
                body()
        else:
            body()

    nc.compile()
    return nc


BUILD_KW = dict(gen="stt", pack="ct")

_CACHE = {}


def _get_nc():
    if "nc" not in _CACHE:
        _CACHE["nc"] = build_bass(**BUILD_KW)
    return _CACHE["nc"]


def run(image, trace=False):
    """image: [16, 2, 512, 512] fp32 -> ([16, 1, 32, 32] fp32, results obj)."""
    image = np.ascontiguousarray(image, dtype=np.float32)
    n = image.shape[0]
    per = n // N_CORES
    flat = image.reshape(n, 2, H * W)
    nc = _get_nc()
    in_maps = [
        {"image": np.ascontiguousarray(flat[per * c:per * (c + 1)])}
        for c in range(N_CORES)
    ]
    res = run_bass_kernel_spmd(nc, in_maps, list(range(N_CORES)), trace=trace)
    out = np.concatenate([res.results[c]["hist"] for c in range(N_CORES)], axis=0)
    return out.astype(np.float32), res


def kernel(image):
    out, _ = run(image)
    return out


# revision 22
# speedup vs baseline: 1.1321x; 1.1321x over previous
"""Trainium2 Bass kernel for nn_ColourHistogram.

Computes, per image, hist = ka^T @ kb where ka/kb are Gaussian
soft-assignment matrices [HW, 32] built from the two channels:
    ka[p, a] = exp(-0.5*((x_p - a/31)/sigma)^2) = exp(-C*(31*x_p - a)^2)
with C = 0.5/(31*sigma)^2.

Strategy (pure data parallel, 2 images per core on 8 cores):
  - pixels live on the 128 SBUF partitions, pixel-columns on the free dim
  - squared-distance planes u_a = (31x - a)^2 are produced on the DVE with
    a per-bin recurrence  u_{a+1} = (u_a + (2a+1)) + (-62x)
  - one ACT pass per bin-group applies exp(-C*u) -> fp16 weights
  - the PE contracts ka^T kb over pixels, accumulating in PSUM
"""

import numpy as np
from contextlib import ExitStack

import jax

try:
    jax.config.update("jax_compilation_cache_dir", "/tmp/jaxcache")
    jax.config.update("jax_persistent_cache_min_compile_time_secs", 1.0)
except Exception:
    pass

import concourse.bass as bass
import concourse.bacc as bacc
import concourse.tile as tile
from concourse import mybir
from concourse.bass_utils import run_bass_kernel_spmd

BINS = 32
SIGMA = 0.05
N_CORES = 8
IMGS = 2          # images per core
H = W = 512
P = 128
COLS = (H * W) // P  # 2048 pixel-columns per channel image
C_EXP = 0.5 / (31.0 * SIGMA) ** 2  # 0.20811654526...

F32 = mybir.dt.float32
F16 = mybir.dt.float16

AF = mybir.ActivationFunctionType
OP = mybir.AluOpType


def build_bass(cols=COLS, slab=512, grp=4, imgs=IMGS, reps=1,
               gen="stt", pack=True, stream_x=False, ubufs=2, kbufs=2,
               act_bins=0):
    """Build the per-core Bass program. Shapes are per-core.

    reps > 1 wraps the computation in a hardware loop so device time can
    be measured as (wall(R) - wall(1)) / (R - 1).
    gen: "stt" uses the fused scalar_tensor_tensor recurrence;
         "native" uses tensor_scalar + tensor_tensor pairs.
    pack: True packs 4 pixel-chunks into [128,128] matmuls (kt layout
          [P, slab, BINS], strided exp writes); False uses one chunk per
          matmul (kt layout [P, BINS, slab], dense exp writes).
    """
    n_slabs = cols // slab
    assert cols % slab == 0 and BINS % grp == 0 and slab % 4 == 0

    nc = bacc.Bacc("TRN2", debug=False)
    image = nc.dram_tensor(
        "image", [imgs, 2, P * cols], F32, kind="ExternalInput"
    )
    hist = nc.dram_tensor(
        "hist", [imgs, 1, BINS, BINS], F32, kind="ExternalOutput"
    )

    with ExitStack() as ctx:
        tc = ctx.enter_context(tile.TileContext(nc))

        xpool = ctx.enter_context(tc.tile_pool(name="x", bufs=1 if not stream_x else 3))
        n2pool = ctx.enter_context(tc.tile_pool(name="n2", bufs=2))
        upool = ctx.enter_context(tc.tile_pool(name="u", bufs=ubufs))
        kpool = ctx.enter_context(tc.tile_pool(name="k", bufs=kbufs))
        pspool = ctx.enter_context(tc.tile_pool(name="ps", bufs=1, space="PSUM"))
        fpool = ctx.enter_context(tc.tile_pool(name="fin", bufs=1))

        bias_na = [None]

        def gen_channel(xs, kt):
            """Fill kt with exp(-C*(31*xs - a)^2) for a = 0..31."""
            if gen == "stt":
                n2 = n2pool.tile([P, slab], F32, tag="n2")
                nc.scalar.mul(n2, xs, -62.0)
            uprev = None
            for g in range(BINS // grp):
                ug = upool.tile([P, grp, slab], F32, tag="u")
                for j in range(grp):
                    a = g * grp + j
                    if a == 0:
                        nc.scalar.activation(
                            out=ug[:, 0, :], in_=xs,
                            func=AF.Square, scale=31.0,
                        )
                        continue
                    if j >= grp - act_bins:
                        # offload the group's last bins to the ACT engine:
                        # u = (31x - a)^2, same scale as the DVE path
                        nc.scalar.activation(
                            out=ug[:, j, :], in_=xs,
                            func=AF.Square, scale=31.0,
                            bias=bias_na[0][:, a:a + 1],
                        )
                        continue
                    usrc = uprev[:, grp - 1, :] if j == 0 else ug[:, j - 1, :]
                    if gen == "stt":
                        # u_a = (u_{a-1} + (2a-1)) + (-62 x)
                        nc.vector.scalar_tensor_tensor(
                            out=ug[:, j, :], in0=usrc,
                            scalar=float(2 * a - 1), in1=n2,
                            op0=OP.add, op1=OP.add,
                        )
                    else:
                        d = n2pool.tile([P, slab], F32, tag="d")
                        nc.vector.tensor_scalar(
                            out=d, in0=xs, scalar1=-62.0,
                            scalar2=float(2 * a - 1),
                            op0=OP.mult, op1=OP.add,
                        )
                        nc.vector.tensor_add(ug[:, j, :], usrc, d)
                if pack is True:
                    out_v = (kt[:, :, g * grp:(g + 1) * grp]
                             .rearrange("p w a -> p a w"))
                else:
                    out_v = kt[:, g * grp:(g + 1) * grp, :]
                nc.scalar.activation(
                    out=out_v, in_=ug, func=AF.Exp, scale=-C_EXP,
                )
                uprev = ug

        def body():
            if act_bins > 0:
                bias_na[0] = fpool.tile([P, BINS], F32, name="biasneg")
                for g in range(BINS // grp):
                    for j in range(grp - act_bins, grp):
                        a = g * grp + j
                        nc.vector.memset(bias_na[0][:, a:a + 1], float(-a))
            # ---- load channel images: x_sb[:, ci, :], ci = img*2 + ch ----
            if not stream_x:
                x_sb = xpool.tile([P, 2 * imgs, cols], F32)
                for img in range(imgs):
                    for ch in range(2):
                        ci = img * 2 + ch
                        src = image[img, ch].rearrange("(p c) -> p c", p=P)
                        nc.sync.dma_start(out=x_sb[:, ci, :], in_=src)

            if pack is True:
                ph = [pspool.tile([P, 4, BINS], F32, name=f"ph{i}")
                      for i in range(imgs)]
            elif pack == "ct":
                ph = [pspool.tile([96, BINS], F32, name=f"ph{i}")
                      for i in range(imgs)]
            else:
                ph = [pspool.tile([BINS, BINS], F32, name=f"ph{i}")
                      for i in range(imgs)]

            kshape = [P, slab, BINS] if pack is True else [P, BINS, slab]
            for img in range(imgs):
                first_mm = True
                for s in range(n_slabs):
                    c0 = s * slab
                    if stream_x:
                        # per-slab channel-pair load (keeps SBUF small)
                        x_sl = xpool.tile([P, 2, slab], F32, tag="xs")
                        src = image[img].rearrange(
                            "c (p w) -> p c w", p=P)[:, :, c0:c0 + slab]
                        nc.sync.dma_start(out=x_sl, in_=src)
                    ksl = []
                    for ch in range(2):
                        ci = img * 2 + ch
                        if stream_x:
                            xs = x_sl[:, ch, :]
                        else:
                            xs = x_sb[:, ci, c0:c0 + slab]  # [128, slab]
                        kt = kpool.tile(kshape, F16, tag=f"k{ch}")
                        ksl.append(kt)
                        gen_channel(xs, kt)

                    if pack is True:
                        # 4 pixel-chunks per LDW+MM pair: [128,128] operands
                        for w in range(0, slab, 4):
                            last = (s == n_slabs - 1) and (w == slab - 4)
                            nc.tensor.matmul(
                                ph[img][:],
                                ksl[0][:, w:w + 4, :],
                                ksl[1][:, w:w + 4, :],
                                start=first_mm, stop=last,
                            )
                            first_mm = False
                    elif pack == "ct":
                        # 4-way column tiling: chunk w runs in col-group
                        # w%4 of the PE array (tile_position auto-derived
                        # from the PSUM slice base partition); the four
                        # 32x32 matmuls execute concurrently
                        for w in range(slab):
                            j = w % 3
                            first = (s == 0) and (w < 3)
                            last = (s == n_slabs - 1) and (w >= slab - 3)
                            nc.tensor.matmul(
                                ph[img][32 * j:32 * (j + 1), :],
                                ksl[0][:, :, w],
                                ksl[1][:, :, w],
                                start=first, stop=last,
                            )
                    else:
                        for w in range(slab):
                            last = (s == n_slabs - 1) and (w == slab - 1)
                            nc.tensor.matmul(
                                ph[img][:],
                                ksl[0][:, :, w],
                                ksl[1][:, :, w],
                                start=first_mm, stop=last,
                            )
                            first_mm = False

            # ---- finalize ----
            for img in range(imgs):
                if pack == "ct":
                    # sum the 3 col-group partials stacked on partitions
                    hsb = fpool.tile([96, BINS], F32, name=f"hsb{img}")
                    nc.vector.tensor_copy(hsb, ph[img][:])
                    dg = fpool.tile([BINS, 3, BINS], F32, name=f"dg{img}")
                    for g in range(3):
                        nc.sync.dma_start(
                            out=dg[:, g, :],
                            in_=hsb[32 * g:32 * (g + 1), :],
                        )
                    t0 = fpool.tile([BINS, BINS], F32, name=f"t0_{img}")
                    t2 = fpool.tile([BINS, BINS], F32, name=f"t2_{img}")
                    nc.vector.tensor_add(t0, dg[:, 0, :], dg[:, 1, :])
                    nc.vector.tensor_add(t2, t0, dg[:, 2, :])
                    nc.sync.dma_start(out=hist[img, 0], in_=t2)
                elif pack is True:
                    # sum the 4 diagonal 32x32 blocks
                    hsb = fpool.tile([P, 4, BINS], F32, name=f"hsb{img}")
                    nc.vector.tensor_copy(hsb, ph[img][:])
                    dg = fpool.tile([BINS, 4, BINS], F32, name=f"dg{img}")
                    for g in range(4):
                        nc.sync.dma_start(
                            out=dg[:, g, :],
                            in_=hsb[32 * g:32 * (g + 1), g, :],
                        )
                    t0 = fpool.tile([BINS, BINS], F32, name=f"t0_{img}")
                    t1 = fpool.tile([BINS, BINS], F32, name=f"t1_{img}")
                    t2 = fpool.tile([BINS, BINS], F32, name=f"t2_{img}")
                    nc.vector.tensor_add(t0, dg[:, 0, :], dg[:, 1, :])
                    nc.vector.tensor_add(t1, dg[:, 2, :], dg[:, 3, :])
                    nc.vector.tensor_add(t2, t0, t1)
                    nc.sync.dma_start(out=hist[img, 0], in_=t2)
                else:
                    t2 = fpool.tile([BINS, BINS], F32, name=f"t2_{img}")
                    nc.vector.tensor_copy(t2, ph[img][:])
                    nc.sync.dma_start(out=hist[img, 0], in_=t2)

        if reps > 1:
            with tc.For_i(0, reps, 1):
                body()
        else:
            body()

    nc.compile()
    return nc


BUILD_KW = dict(gen="stt", pack="ct")

_CACHE = {}


def _get_nc():
    if "nc" not in _CACHE:
        _CACHE["nc"] = build_bass(**BUILD_KW)
    return _CACHE["nc"]


def run(image, trace=False):
    """image: [16, 2, 512, 512] fp32 -> ([16, 1, 32, 32] fp32, results obj)."""
    image = np.ascontiguousarray(image, dtype=np.float32)
    n = image.shape[0]
    per = n // N_CORES
    flat = image.reshape(n, 2, H * W)
    nc = _get_nc()
    in_maps = [
        {"image": np.ascontiguousarray(flat[per * c:per * (c + 1)])}
        for c in range(N_CORES)
    ]
    res = run_bass_kernel_spmd(nc, in_maps, list(range(N_CORES)), trace=trace)
    out = np.concatenate([res.results[c]["hist"] for c in range(N_CORES)], axis=0)
    return out.astype(np.float32), res


def kernel(image):
    out, _ = run(image)
    return out
